# revision 25
# baseline (speedup 1.0000x reference)
"""ComplexRNN Trainium2 kernel — layer-interleaved scan.

Problem: 2-layer complex-valued tanh RNN.
  B=8, T=4096, FEA=512 (256 complex in), H_C=256 complex hidden.
  Per layer: wx = complexLinear(x, W) (big GEMM over all time steps),
  then sequential scan h_t = tanh(wx_t + complexLinear(h_{t-1}, U)).

Sharding: data-parallel over batch. 8 batch rows -> 8 NeuronCores, one
row per core; weights replicated. Each core runs both layers for its
row, with layer 1 lagging layer 0 by one block so the two scans form
two independent dependency chains that interleave on every engine.

Per-step structure (the trace-driven redesign vs the phase-split
baseline):
  - State kept as 8 columns per step: (hr,hi,nhr,nhi) for both 128-
    chunks, where nh* = tanh(-z*) = -h*.  One ACT instruction computes
    all 8 from an 8-column PSUM holding (z, -z); no separate negate /
    copy instructions (tanh is odd).
  - wx_t is injected into PSUM by two identity matmuls (+I, -I
    stationary) at the head of the accumulation group — they depend
    only on the prefetched wx block, so they run while the previous
    step's tanh is still in flight; the DVE add of the baseline is
    gone from the critical cycle.
  - Critical cycle per step: last U-matmul -> ACT tanh (PSUM->SBUF)
    -> next step's U-matmuls. 14 LDW+MM pairs/step issue at ~30ns.
  - Layer-1 wx is produced by an inline GEMM on each 32-step block
    (rhs = layer-0 h staging tile), so no full-T wx1 buffer exists and
    no cross-phase dependency stalls the loop.

Complex matmul mapping (per layer; ur/ui are 256x256 in 2x2 chunks of
128).  Column order (chunk, sign, re/im) makes every matmul's output a
CONTIGUOUS psum slice (matmul out APs must flatten to 2D and the psum
has_written model wants contiguous writes):
  psum col = j*4 + s*2 + ri : (zr0, zi0, nzr0, nzi0, zr1, zi1, ...)
  h8 col   = k*4 + s*2 + ri : (hr0, hi0, nhr0, nhi0, hr1, hi1, ...)
wx is precomputed with both signs in the same order:
  wx8 col  = j*4 + s*2 + ri : (+wxr0, +wxi0, -wxr0, -wxi0, +wxr1, ...)
Per step (13 matmuls, one accumulation group):
  I @ wx8_t -> ps[0:8]                                    [N=8]
  ur[k,j] @ h8[4k:4k+4] -> ps[4j:4j+4]                    [N=4]
  ui[k,j] @ (nhi,hr)_k [4k+3::-3] -> (zr,zi)_j [4j:4j+2]  [N=2]
  ui[k,j] @ (hi,nhr)_k [4k+1:4k+3] -> (nzr,nzi)_j         [N=2]
"""

import sys

sys.path.insert(0, "/opt/trn_rl_repo")

import numpy as np

import concourse.bass as bass
import concourse.bacc as bacc
import concourse.mybir as mybir
import concourse.tile as tile
from concourse.bass import ds
from concourse.bass_utils import run_bass_kernel_spmd
from concourse.masks import make_identity

F32 = mybir.dt.float32
F16 = mybir.dt.float16

B = 8
T = 4096
FEA = 512
HC = 256  # complex hidden units; real state width = 2*HC = 512
NCORES = 8

Tanh = mybir.ActivationFunctionType.Tanh
Identity = mybir.ActivationFunctionType.Identity

CMAP = [0, 2, 1, 3]  # W f-chunk (r0,r1,i0,i1) -> h col block (hr0,hi0,hr1,hi1)
OPERM = [0, 2, 1, 3]  # ht col block -> output column block


def build_program(t_len=T, unroll=256, scan_dt=F16, gemm_dt=F16):
    """Build the SPMD Bass program for one core (one batch row)."""
    nc = bacc.Bacc("TRN2", target_bir_lowering=False)

    x_d = nc.declare_dram_parameter("x", [t_len, FEA], F32, isOutput=False)
    w_d = [
        nc.declare_dram_parameter(f"w{l}", [128, 4 * 512], gemm_dt, isOutput=False)
        for l in range(2)
    ]
    u_d = [
        nc.declare_dram_parameter(f"u{l}", [128, 8 * 128], scan_dt, isOutput=False)
        for l in range(2)
    ]
    b_d = [
        nc.declare_dram_parameter(f"b{l}", [128, 4], F32, isOutput=False)
        for l in range(2)
    ]
    out_d = nc.declare_dram_parameter("out", [t_len, FEA], F32, isOutput=True)

    nblk = t_len // unroll
    assert nblk % 2 == 0
    n_ttile = t_len // 512  # GEMM0 moving-dim tiles
    n_ptile = t_len // 128  # transpose tiles

    with tile.TileContext(nc) as tc:
        with (
            tc.tile_pool(name="consts", bufs=1) as consts,
            tc.tile_pool(name="big", bufs=1) as bigp,
        ):
            # ---- constants ----
            w_sb = [consts.tile([128, 4 * 512], gemm_dt, tag=f"w{l}", name=f"w{l}sb") for l in range(2)]
            u_sb = [consts.tile([128, 8 * 128], scan_dt, tag=f"u{l}", name=f"u{l}sb") for l in range(2)]
            b_sb = [consts.tile([128, 4], F32, tag=f"b{l}", name=f"b{l}sb") for l in range(2)]
            nb_sb = [consts.tile([128, 4], F32, tag=f"nb{l}", name=f"nb{l}sb") for l in range(2)]
            for l in range(2):
                nc.sync.dma_start(out=w_sb[l][:], in_=w_d[l][:])
                nc.sync.dma_start(out=u_sb[l][:], in_=u_d[l][:])
                nc.sync.dma_start(out=b_sb[l][:], in_=b_d[l][:])
                nc.vector.tensor_scalar_mul(nb_sb[l][:], b_sb[l][:], -1.0)
            ident32 = consts.tile([128, 128], F32, tag="id32")
            make_identity(nc, ident32)
            identp = consts.tile([128, 128], scan_dt, tag="idp")
            make_identity(nc, identp)

            # ---- big tensors ----
            xt = bigp.tile([128, 4, t_len], gemm_dt, tag="xt")
            # wx0 padded by one block: last prefetch reads one block past end
            wx0 = bigp.tile([128, 8, t_len + unroll], gemm_dt, tag="wx0")
            # ht1 padded by one block at the FRONT (layer-1 lags by one
            # block; its first in-loop block is garbage-free zeros)
            ht1 = bigp.tile([128, 2, 2, t_len + unroll], scan_dt, tag="ht1")

            # ---- phase B: transpose x into XT ----
            with (
                tc.tile_pool(name="xstage", bufs=3) as xstage,
                tc.tile_pool(name="pst", bufs=4, space="PSUM") as pst,
            ):
                for tt in range(n_ptile):
                    xtile = xstage.tile([128, FEA], F32, tag="xin")
                    nc.sync.dma_start(
                        out=xtile[:], in_=x_d[tt * 128 : (tt + 1) * 128, :]
                    )
                    for fc in range(4):
                        ps = pst.tile([128, 128], F32, tag="tr")
                        nc.tensor.transpose(
                            ps[:], xtile[:, fc * 128 : (fc + 1) * 128], ident32[:]
                        )
                        nc.vector.tensor_copy(
                            out=xt[:, fc, tt * 128 : (tt + 1) * 128], in_=ps[:]
                        )

            # ---- phase C: GEMM layer 0 -> wx0 (f16, bias folded) ----
            with tc.tile_pool(name="psg0", bufs=2, space="PSUM") as psg0:
                for jb in range(4):
                    for tt in range(n_ttile):
                        ps = psg0.tile([128, 512], F32, tag="g0")
                        for fc in range(4):
                            nc.tensor.matmul(
                                ps[:],
                                w_sb[0][:, fc * 512 + jb * 128 : fc * 512 + (jb + 1) * 128],
                                xt[:, fc, tt * 512 : (tt + 1) * 512],
                                start=(fc == 0),
                                stop=(fc == 3),
                            )
                        j, ri = jb // 2, jb % 2
                        nc.scalar.activation(
                            wx0[:, j * 4 + ri, tt * 512 : (tt + 1) * 512],
                            ps[:],
                            Identity,
                            bias=b_sb[0][:, jb : jb + 1],
                        )
                        nc.scalar.activation(
                            wx0[:, j * 4 + 2 + ri, tt * 512 : (tt + 1) * 512],
                            ps[:],
                            Identity,
                            scale=-1.0,
                            bias=nb_sb[0][:, jb : jb + 1],
                        )

            # ---- phase D: fused dual-layer scan ----
            wx0_v = wx0.rearrange("p a (n u) -> p a n u", u=unroll)
            ht1_v = ht1.rearrange("p a b (n u) -> p a b n u", u=unroll)
            nc.vector.memset(wx0[:, :, t_len:], 0.0)

            h8r = [
                [
                    consts.tile([128, 8], scan_dt, tag=f"h8_{l}_{u}", name=f"h8_{l}_{u}")
                    for u in range(unroll)
                ]
                for l in range(2)
            ]
            for l in range(2):
                for t8 in h8r[l]:
                    nc.vector.memset(t8[:], 0.0)
            wxblk0 = [
                consts.tile([128, 8, 1, unroll], gemm_dt, tag=f"wxb{s}", name=f"wxb{s}")
                for s in range(2)
            ]
            wx1r = [
                consts.tile([128, 8, unroll], gemm_dt, tag=f"wx1r{s}", name=f"wx1r{s}")
                for s in range(2)
            ]
            nc.vector.memset(wx1r[1][:], 0.0)
            hblk0 = [
                consts.tile([128, 2, 2, 1, unroll], scan_dt, tag=f"hb0{s}", name=f"hb0{s}")
                for s in range(2)
            ]
            hblk1 = [
                consts.tile([128, 2, 2, 1, unroll], scan_dt, tag=f"hb1{s}", name=f"hb1{s}")
                for s in range(2)
            ]

            def uchunk(l, v, k, j):
                o = ((v * 2 + k) * 2 + j) * 128
                return u_sb[l][:, o : o + 128]

            with tc.tile_pool(name="psscan", bufs=2, space="PSUM") as psscan:

                def half(l, u, j, wx_ap, hp, h8):
                    # one output chunk j -> its own psum bank; j-major order
                    # lets tanh(j=0) overlap the j=1 matmuls of the same step
                    ps = psscan.tile([128, 4], F32, tag=f"ps{l}{j}", name=f"ps{l}{j}")
                    nc.tensor.matmul(
                        ps[:], identp[:], wx_ap[:, 4 * j : 4 * j + 4],
                        start=True, stop=False,
                    )
                    for k in range(2):
                        nc.tensor.matmul(
                            ps[:], uchunk(l, 0, k, j), hp[:, 4 * k : 4 * k + 4],
                            start=False, stop=False,
                        )
                    for k in range(2):
                        nc.tensor.matmul(
                            ps[:, 0:2], uchunk(l, 1, k, j),
                            hp[:, 4 * k + 3 :: -3][:, :2],
                            start=False, stop=False,
                        )
                        nc.tensor.matmul(
                            ps[:, 2:4], uchunk(l, 1, k, j),
                            hp[:, 4 * k + 1 : 4 * k + 3],
                            start=False, stop=(k == 1),
                        )
                    nc.scalar.activation(h8[:, 4 * j : 4 * j + 4], ps[:], Tanh)

                def step(l, u, wx_ap, hb):
                    hp = h8r[l][(u - 1) % unroll]
                    h8 = h8r[l][u]
                    half(l, u, 0, wx_ap, hp, h8)
                    half(l, u, 1, wx_ap, hp, h8)
                    nc.vector.tensor_copy(
                        out=hb[:, :, :, 0, u : u + 1],
                        in_=h8.rearrange("p (k s r) -> p k s r", k=2, s=2)[:, :, 0, :, None],
                    )

                def gemm1(s):
                    for jb in range(4):
                        psg = psscan.tile([128, unroll], F32, tag="ps00", name="g1")
                        for fc in range(4):
                            nc.tensor.matmul(
                                psg[:],
                                w_sb[1][:, fc * 512 + jb * 128 : fc * 512 + (jb + 1) * 128],
                                hblk0[s][:, CMAP[fc] // 2, CMAP[fc] % 2, 0, :],
                                start=(fc == 0),
                                stop=(fc == 3),
                            )
                        j, ri = jb // 2, jb % 2
                        nc.vector.tensor_scalar(
                            wx1r[s][:, j * 4 + ri, :], psg[:],
                            b_sb[1][:, jb : jb + 1], None,
                            mybir.AluOpType.add,
                        )
                        nc.vector.tensor_scalar(
                            wx1r[s][:, j * 4 + 2 + ri, :], psg[:],
                            b_sb[1][:, jb : jb + 1], -1.0,
                            mybir.AluOpType.add,
                            mybir.AluOpType.mult,
                        )

                # preload L0 block 0
                nc.sync.dma_start(out=wxblk0[0][:], in_=wx0_v[:, :, 0:1, :])
                with tc.For_i(0, nblk // 2, 1, hint_engines=(mybir.EngineType.PE,)) as iv:
                    # ---- half s=0: L0 block 2i, L1 block 2i-1 ----
                    nc.sync.dma_start(
                        out=wxblk0[1][:], in_=wx0_v[:, :, ds(iv * 2 + 1, 1), :]
                    )
                    for u in range(unroll):
                        step(0, u, wxblk0[0][:, :, 0, u], hblk0[0])
                        step(1, u, wx1r[1][:, :, u], hblk1[0])
                    gemm1(0)
                    nc.sync.dma_start(
                        out=ht1_v[:, :, :, ds(iv * 2, 1), :], in_=hblk1[0][:]
                    )
                    # ---- half s=1: L0 block 2i+1, L1 block 2i ----
                    nc.sync.dma_start(
                        out=wxblk0[0][:], in_=wx0_v[:, :, ds(iv * 2 + 2, 1), :]
                    )
                    for u in range(unroll):
                        step(0, u, wxblk0[1][:, :, 0, u], hblk0[1])
                        step(1, u, wx1r[0][:, :, u], hblk1[1])
                    gemm1(1)
                    nc.sync.dma_start(
                        out=ht1_v[:, :, :, ds(iv * 2 + 1, 1), :], in_=hblk1[1][:]
                    )

                # ---- phase E: epilogue, L1 last block ----
                for u in range(unroll):
                    step(1, u, wx1r[1][:, :, u], hblk1[0])
                nc.sync.dma_start(
                    out=ht1_v[:, :, :, nblk : nblk + 1, :], in_=hblk1[0][:]
                )

            # ---- phase F: transpose ht1 -> out ----
            with (
                tc.tile_pool(name="ostage", bufs=3) as ostage,
                tc.tile_pool(name="pso", bufs=4, space="PSUM") as pso,
            ):
                for tt in range(n_ptile):
                    otile = ostage.tile([128, FEA], F32, tag="ot")
                    for c in range(4):
                        ps = pso.tile([128, 128], scan_dt, tag="tro")
                        nc.tensor.transpose(
                            ps[:],
                            ht1[:, c // 2, c % 2, unroll + tt * 128 : unroll + (tt + 1) * 128],
                            identp[:],
                        )
                        nc.vector.tensor_copy(
                            out=otile[:, OPERM[c] * 128 : (OPERM[c] + 1) * 128],
                            in_=ps[:],
                        )
                    nc.sync.dma_start(
                        out=out_d[tt * 128 : (tt + 1) * 128, :], in_=otile[:]
                    )

    nc.compile()
    return nc


def prep_weights(wr, wi, wbr, wbi, ur, ui, ubr, ubi, scan_np, gemm_np):
    """Host-side packing of one layer's weights into the kernel layouts."""
    in_c = wr.shape[0]
    wfull = np.block([[wr, wi], [-wi, wr]]).astype(np.float32)  # [2*in_c, 512]
    colperm = np.concatenate(
        [np.arange(0, 128), np.arange(256, 384), np.arange(128, 256), np.arange(384, 512)]
    )
    wperm = wfull[:, colperm]  # [2*in_c, 512]
    nf = 2 * in_c
    assert nf == 512
    w_sb = (
        wperm.reshape(4, 128, 512).transpose(1, 0, 2).reshape(128, 4 * 512)
    ).astype(gemm_np)
    bsum = np.concatenate([wbr + ubr, wbi + ubi]).astype(np.float32)[colperm]
    b_sb = np.ascontiguousarray(bsum.reshape(4, 128).T).astype(np.float32)
    # u chunks: [(v*2+k)*2+j]*128 offset; u_v[k*128+p, j*128+m]
    u_sb = (
        np.stack([ur, ui])  # [2, 256, 256]
        .reshape(2, 2, 128, 2, 128)  # v, k, p, j, m
        .transpose(2, 0, 1, 3, 4)  # p, v, k, j, m
        .reshape(128, 8 * 128)
    ).astype(scan_np)
    return w_sb, u_sb, b_sb


_PROG_CACHE = {}


def _get_program():
    key = "main"
    if key not in _PROG_CACHE:
        _PROG_CACHE[key] = build_program()
    return _PROG_CACHE[key]


def _make_in_maps(inputs, scan_np=np.float16, gemm_np=np.float16):
    x = np.asarray(inputs["x"], dtype=np.float32)
    shared = {}
    for l in range(2):
        w_sb, u_sb, b_sb = prep_weights(
            np.asarray(inputs[f"l{l}_wr"], np.float32),
            np.asarray(inputs[f"l{l}_wi"], np.float32),
            np.asarray(inputs[f"l{l}_wbr"], np.float32),
            np.asarray(inputs[f"l{l}_wbi"], np.float32),
            np.asarray(inputs[f"l{l}_ur"], np.float32),
            np.asarray(inputs[f"l{l}_ui"], np.float32),
            np.asarray(inputs[f"l{l}_ubr"], np.float32),
            np.asarray(inputs[f"l{l}_ubi"], np.float32),
            scan_np,
            gemm_np,
        )
        shared[f"w{l}"] = w_sb
        shared[f"u{l}"] = u_sb
        shared[f"b{l}"] = b_sb
    in_maps = []
    for b in range(B):
        m = dict(shared)
        m["x"] = np.ascontiguousarray(x[b])
        in_maps.append(m)
    return in_maps


def run(inputs, trace=False):
    nc = _get_program()
    in_maps = _make_in_maps(inputs)
    res = run_bass_kernel_spmd(nc, in_maps, list(range(NCORES)), trace=trace)
    out = np.stack([res.results[b]["out"] for b in range(B)], axis=0)
    return out.astype(np.float32), res


def kernel(**inputs):
    out, _ = run(inputs, trace=False)
    return out


# revision 27
# speedup vs baseline: 1.0103x; 1.0103x over previous
"""ComplexRNN Trainium2 kernel — layer-interleaved scan.

Problem: 2-layer complex-valued tanh RNN.
  B=8, T=4096, FEA=512 (256 complex in), H_C=256 complex hidden.
  Per layer: wx = complexLinear(x, W) (big GEMM over all time steps),
  then sequential scan h_t = tanh(wx_t + complexLinear(h_{t-1}, U)).

Sharding: data-parallel over batch. 8 batch rows -> 8 NeuronCores, one
row per core; weights replicated. Each core runs both layers for its
row, with layer 1 lagging layer 0 by one block so the two scans form
two independent dependency chains that interleave on every engine.

Per-step structure (the trace-driven redesign vs the phase-split
baseline):
  - State kept as 8 columns per step: (hr,hi,nhr,nhi) for both 128-
    chunks, where nh* = tanh(-z*) = -h*.  One ACT instruction computes
    all 8 from an 8-column PSUM holding (z, -z); no separate negate /
    copy instructions (tanh is odd).
  - wx_t is injected into PSUM by two identity matmuls (+I, -I
    stationary) at the head of the accumulation group — they depend
    only on the prefetched wx block, so they run while the previous
    step's tanh is still in flight; the DVE add of the baseline is
    gone from the critical cycle.
  - Critical cycle per step: last U-matmul -> ACT tanh (PSUM->SBUF)
    -> next step's U-matmuls. 14 LDW+MM pairs/step issue at ~30ns.
  - Layer-1 wx is produced by an inline GEMM on each 32-step block
    (rhs = layer-0 h staging tile), so no full-T wx1 buffer exists and
    no cross-phase dependency stalls the loop.

Complex matmul mapping (per layer; ur/ui are 256x256 in 2x2 chunks of
128).  Column order (chunk, sign, re/im) makes every matmul's output a
CONTIGUOUS psum slice (matmul out APs must flatten to 2D and the psum
has_written model wants contiguous writes):
  psum col = j*4 + s*2 + ri : (zr0, zi0, nzr0, nzi0, zr1, zi1, ...)
  h8 col   = k*4 + s*2 + ri : (hr0, hi0, nhr0, nhi0, hr1, hi1, ...)
wx is precomputed with both signs in the same order:
  wx8 col  = j*4 + s*2 + ri : (+wxr0, +wxi0, -wxr0, -wxi0, +wxr1, ...)
Per step (13 matmuls, one accumulation group):
  I @ wx8_t -> ps[0:8]                                    [N=8]
  ur[k,j] @ h8[4k:4k+4] -> ps[4j:4j+4]                    [N=4]
  ui[k,j] @ (nhi,hr)_k [4k+3::-3] -> (zr,zi)_j [4j:4j+2]  [N=2]
  ui[k,j] @ (hi,nhr)_k [4k+1:4k+3] -> (nzr,nzi)_j         [N=2]
"""

import sys

sys.path.insert(0, "/opt/trn_rl_repo")

import numpy as np

import concourse.bass as bass
import concourse.bacc as bacc
import concourse.mybir as mybir
import concourse.tile as tile
from concourse.bass import ds
from concourse.bass_utils import run_bass_kernel_spmd
from concourse.masks import make_identity

F32 = mybir.dt.float32
F16 = mybir.dt.float16

B = 8
T = 4096
FEA = 512
HC = 256  # complex hidden units; real state width = 2*HC = 512
NCORES = 8

Tanh = mybir.ActivationFunctionType.Tanh
Identity = mybir.ActivationFunctionType.Identity

CMAP = [0, 2, 1, 3]  # W f-chunk (r0,r1,i0,i1) -> h col block (hr0,hi0,hr1,hi1)
OPERM = [0, 2, 1, 3]  # ht col block -> output column block


def build_program(t_len=T, unroll=128, scan_dt=F16, gemm_dt=F16):
    """Build the SPMD Bass program for one core (one batch row)."""
    nc = bacc.Bacc("TRN2", target_bir_lowering=False)

    x_d = nc.declare_dram_parameter("x", [t_len, FEA], F32, isOutput=False)
    w_d = [
        nc.declare_dram_parameter(f"w{l}", [128, 4 * 512], gemm_dt, isOutput=False)
        for l in range(2)
    ]
    u_d = [
        nc.declare_dram_parameter(f"u{l}", [128, 8 * 128], scan_dt, isOutput=False)
        for l in range(2)
    ]
    b_d = [
        nc.declare_dram_parameter(f"b{l}", [128, 4], F32, isOutput=False)
        for l in range(2)
    ]
    out_d = nc.declare_dram_parameter("out", [t_len, FEA], F32, isOutput=True)

    nblk = t_len // unroll
    assert nblk % 2 == 0
    n_ttile = t_len // 512  # GEMM0 moving-dim tiles
    n_ptile = t_len // 128  # transpose tiles

    with tile.TileContext(nc) as tc:
        with (
            tc.tile_pool(name="consts", bufs=1) as consts,
            tc.tile_pool(name="big", bufs=1) as bigp,
        ):
            # ---- constants ----
            w_sb = [consts.tile([128, 4 * 512], gemm_dt, tag=f"w{l}", name=f"w{l}sb") for l in range(2)]
            u_sb = [consts.tile([128, 8 * 128], scan_dt, tag=f"u{l}", name=f"u{l}sb") for l in range(2)]
            b_sb = [consts.tile([128, 4], F32, tag=f"b{l}", name=f"b{l}sb") for l in range(2)]
            nb_sb = [consts.tile([128, 4], F32, tag=f"nb{l}", name=f"nb{l}sb") for l in range(2)]
            for l in range(2):
                nc.sync.dma_start(out=w_sb[l][:], in_=w_d[l][:])
                nc.sync.dma_start(out=u_sb[l][:], in_=u_d[l][:])
                nc.sync.dma_start(out=b_sb[l][:], in_=b_d[l][:])
                nc.vector.tensor_scalar_mul(nb_sb[l][:], b_sb[l][:], -1.0)
            ident32 = consts.tile([128, 128], F32, tag="id32")
            make_identity(nc, ident32)
            identp = consts.tile([128, 128], scan_dt, tag="idp")
            make_identity(nc, identp)

            # ---- big tensors ----
            xt = bigp.tile([128, 4, t_len], gemm_dt, tag="xt")
            # wx0 padded by one block: last prefetch reads one block past end
            wx0 = bigp.tile([128, 8, t_len + unroll], gemm_dt, tag="wx0")
            # ht1 padded by one block at the FRONT (layer-1 lags by one
            # block; its first in-loop block is garbage-free zeros)
            ht1 = bigp.tile([128, 2, 2, t_len + unroll], scan_dt, tag="ht1")

            # ---- phase B: transpose x into XT ----
            with (
                tc.tile_pool(name="xstage", bufs=3) as xstage,
                tc.tile_pool(name="pst", bufs=4, space="PSUM") as pst,
            ):
                for tt in range(n_ptile):
                    xtile = xstage.tile([128, FEA], F32, tag="xin")
                    nc.sync.dma_start(
                        out=xtile[:], in_=x_d[tt * 128 : (tt + 1) * 128, :]
                    )
                    for fc in range(4):
                        ps = pst.tile([128, 128], F32, tag="tr")
                        nc.tensor.transpose(
                            ps[:], xtile[:, fc * 128 : (fc + 1) * 128], ident32[:]
                        )
                        nc.vector.tensor_copy(
                            out=xt[:, fc, tt * 128 : (tt + 1) * 128], in_=ps[:]
                        )

            # ---- phase C: GEMM layer 0 -> wx0 (f16, bias folded) ----
            with tc.tile_pool(name="psg0", bufs=2, space="PSUM") as psg0:
                for jb in range(4):
                    for tt in range(n_ttile):
                        ps = psg0.tile([128, 512], F32, tag="g0")
                        for fc in range(4):
                            nc.tensor.matmul(
                                ps[:],
                                w_sb[0][:, fc * 512 + jb * 128 : fc * 512 + (jb + 1) * 128],
                                xt[:, fc, tt * 512 : (tt + 1) * 512],
                                start=(fc == 0),
                                stop=(fc == 3),
                            )
                        j, ri = jb // 2, jb % 2
                        nc.scalar.activation(
                            wx0[:, j * 4 + ri, tt * 512 : (tt + 1) * 512],
                            ps[:],
                            Identity,
                            bias=b_sb[0][:, jb : jb + 1],
                        )
                        nc.scalar.activation(
                            wx0[:, j * 4 + 2 + ri, tt * 512 : (tt + 1) * 512],
                            ps[:],
                            Identity,
                            scale=-1.0,
                            bias=nb_sb[0][:, jb : jb + 1],
                        )

            # ---- phase D: fused dual-layer scan ----
            wx0_v = wx0.rearrange("p a (n u) -> p a n u", u=unroll)
            ht1_v = ht1.rearrange("p a b (n u) -> p a b n u", u=unroll)
            nc.vector.memset(wx0[:, :, t_len:], 0.0)

            h8r = [
                [
                    consts.tile([128, 8], scan_dt, tag=f"h8_{l}_{u}", name=f"h8_{l}_{u}")
                    for u in range(unroll)
                ]
                for l in range(2)
            ]
            for l in range(2):
                for t8 in h8r[l]:
                    nc.vector.memset(t8[:], 0.0)
            wxblk0 = [
                consts.tile([128, 8, 1, unroll], gemm_dt, tag=f"wxb{s}", name=f"wxb{s}")
                for s in range(2)
            ]
            wx1r = [
                consts.tile([128, 8, unroll], gemm_dt, tag=f"wx1r{s}", name=f"wx1r{s}")
                for s in range(2)
            ]
            nc.vector.memset(wx1r[1][:], 0.0)
            hblk0 = [
                consts.tile([128, 2, 2, 1, unroll], scan_dt, tag=f"hb0{s}", name=f"hb0{s}")
                for s in range(2)
            ]
            hblk1 = [
                consts.tile([128, 2, 2, 1, unroll], scan_dt, tag=f"hb1{s}", name=f"hb1{s}")
                for s in range(2)
            ]

            def uchunk(l, v, k, j):
                o = ((v * 2 + k) * 2 + j) * 128
                return u_sb[l][:, o : o + 128]

            with tc.tile_pool(name="psscan", bufs=2, space="PSUM") as psscan:

                def half(l, u, j, wx_ap, hp, h8):
                    # one output chunk j -> its own psum bank; j-major order
                    # lets tanh(j=0) overlap the j=1 matmuls of the same step
                    ps = psscan.tile([128, 4], F32, tag=f"ps{l}{j}", name=f"ps{l}{j}")
                    nc.tensor.matmul(
                        ps[:], identp[:], wx_ap[:, 4 * j : 4 * j + 4],
                        start=True, stop=False,
                    )
                    # k-ordered: all k=0 consumers first, so the k=1 half's
                    # tanh (ACT_j1 of the previous step) gets extra slack
                    for k in range(2):
                        nc.tensor.matmul(
                            ps[:], uchunk(l, 0, k, j), hp[:, 4 * k : 4 * k + 4],
                            start=False, stop=False,
                        )
                        nc.tensor.matmul(
                            ps[:, 0:2], uchunk(l, 1, k, j),
                            hp[:, 4 * k + 3 :: -3][:, :2],
                            start=False, stop=False,
                        )
                        nc.tensor.matmul(
                            ps[:, 2:4], uchunk(l, 1, k, j),
                            hp[:, 4 * k + 1 : 4 * k + 3],
                            start=False, stop=(k == 1),
                        )
                    nc.scalar.activation(h8[:, 4 * j : 4 * j + 4], ps[:], Tanh)

                def step(l, u, wx_ap, hb):
                    hp = h8r[l][(u - 1) % unroll]
                    h8 = h8r[l][u]
                    half(l, u, 0, wx_ap, hp, h8)
                    half(l, u, 1, wx_ap, hp, h8)
                    nc.vector.tensor_copy(
                        out=hb[:, :, :, 0, u : u + 1],
                        in_=h8.rearrange("p (k s r) -> p k s r", k=2, s=2)[:, :, 0, :, None],
                    )

                def gemm1(s):
                    for jb in range(4):
                        psg = psscan.tile([128, unroll], F32, tag="ps00", name="g1")
                        for fc in range(4):
                            nc.tensor.matmul(
                                psg[:],
                                w_sb[1][:, fc * 512 + jb * 128 : fc * 512 + (jb + 1) * 128],
                                hblk0[s][:, CMAP[fc] // 2, CMAP[fc] % 2, 0, :],
                                start=(fc == 0),
                                stop=(fc == 3),
                            )
                        j, ri = jb // 2, jb % 2
                        nc.vector.tensor_scalar(
                            wx1r[s][:, j * 4 + ri, :], psg[:],
                            b_sb[1][:, jb : jb + 1], None,
                            mybir.AluOpType.add,
                        )
                        nc.vector.tensor_scalar(
                            wx1r[s][:, j * 4 + 2 + ri, :], psg[:],
                            b_sb[1][:, jb : jb + 1], -1.0,
                            mybir.AluOpType.add,
                            mybir.AluOpType.mult,
                        )

                # preload L0 block 0
                nc.sync.dma_start(out=wxblk0[0][:], in_=wx0_v[:, :, 0:1, :])
                with tc.For_i(0, nblk // 2, 1, hint_engines=(mybir.EngineType.PE,)) as iv:
                    # ---- half s=0: L0 block 2i, L1 block 2i-1 ----
                    nc.sync.dma_start(
                        out=wxblk0[1][:], in_=wx0_v[:, :, ds(iv * 2 + 1, 1), :]
                    )
                    for u in range(unroll):
                        step(0, u, wxblk0[0][:, :, 0, u], hblk0[0])
                        step(1, u, wx1r[1][:, :, u], hblk1[0])
                    gemm1(0)
                    nc.sync.dma_start(
                        out=ht1_v[:, :, :, ds(iv * 2, 1), :], in_=hblk1[0][:]
                    )
                    # ---- half s=1: L0 block 2i+1, L1 block 2i ----
                    nc.sync.dma_start(
                        out=wxblk0[0][:], in_=wx0_v[:, :, ds(iv * 2 + 2, 1), :]
                    )
                    for u in range(unroll):
                        step(0, u, wxblk0[1][:, :, 0, u], hblk0[1])
                        step(1, u, wx1r[0][:, :, u], hblk1[1])
                    gemm1(1)
                    nc.sync.dma_start(
                        out=ht1_v[:, :, :, ds(iv * 2 + 1, 1), :], in_=hblk1[1][:]
                    )

                # ---- phase E: epilogue, L1 last block ----
                for u in range(unroll):
                    step(1, u, wx1r[1][:, :, u], hblk1[0])
                nc.sync.dma_start(
                    out=ht1_v[:, :, :, nblk : nblk + 1, :], in_=hblk1[0][:]
                )

            # ---- phase F: transpose ht1 -> out ----
            with (
                tc.tile_pool(name="ostage", bufs=3) as ostage,
                tc.tile_pool(name="pso", bufs=4, space="PSUM") as pso,
            ):
                for tt in range(n_ptile):
                    otile = ostage.tile([128, FEA], F32, tag="ot")
                    for c in range(4):
                        ps = pso.tile([128, 128], scan_dt, tag="tro")
                        nc.tensor.transpose(
                            ps[:],
                            ht1[:, c // 2, c % 2, unroll + tt * 128 : unroll + (tt + 1) * 128],
                            identp[:],
                        )
                        nc.vector.tensor_copy(
                            out=otile[:, OPERM[c] * 128 : (OPERM[c] + 1) * 128],
                            in_=ps[:],
                        )
                    nc.sync.dma_start(
                        out=out_d[tt * 128 : (tt + 1) * 128, :], in_=otile[:]
                    )

    nc.compile()
    return nc


def prep_weights(wr, wi, wbr, wbi, ur, ui, ubr, ubi, scan_np, gemm_np):
    """Host-side packing of one layer's weights into the kernel layouts."""
    in_c = wr.shape[0]
    wfull = np.block([[wr, wi], [-wi, wr]]).astype(np.float32)  # [2*in_c, 512]
    colperm = np.concatenate(
        [np.arange(0, 128), np.arange(256, 384), np.arange(128, 256), np.arange(384, 512)]
    )
    wperm = wfull[:, colperm]  # [2*in_c, 512]
    nf = 2 * in_c
    assert nf == 512
    w_sb = (
        wperm.reshape(4, 128, 512).transpose(1, 0, 2).reshape(128, 4 * 512)
    ).astype(gemm_np)
    bsum = np.concatenate([wbr + ubr, wbi + ubi]).astype(np.float32)[colperm]
    b_sb = np.ascontiguousarray(bsum.reshape(4, 128).T).astype(np.float32)
    # u chunks: [(v*2+k)*2+j]*128 offset; u_v[k*128+p, j*128+m]
    u_sb = (
        np.stack([ur, ui])  # [2, 256, 256]
        .reshape(2, 2, 128, 2, 128)  # v, k, p, j, m
        .transpose(2, 0, 1, 3, 4)  # p, v, k, j, m
        .reshape(128, 8 * 128)
    ).astype(scan_np)
    return w_sb, u_sb, b_sb


_PROG_CACHE = {}


def _get_program():
    key = "main"
    if key not in _PROG_CACHE:
        _PROG_CACHE[key] = build_program()
    return _PROG_CACHE[key]


def _make_in_maps(inputs, scan_np=np.float16, gemm_np=np.float16):
    x = np.asarray(inputs["x"], dtype=np.float32)
    shared = {}
    for l in range(2):
        w_sb, u_sb, b_sb = prep_weights(
            np.asarray(inputs[f"l{l}_wr"], np.float32),
            np.asarray(inputs[f"l{l}_wi"], np.float32),
            np.asarray(inputs[f"l{l}_wbr"], np.float32),
            np.asarray(inputs[f"l{l}_wbi"], np.float32),
            np.asarray(inputs[f"l{l}_ur"], np.float32),
            np.asarray(inputs[f"l{l}_ui"], np.float32),
            np.asarray(inputs[f"l{l}_ubr"], np.float32),
            np.asarray(inputs[f"l{l}_ubi"], np.float32),
            scan_np,
            gemm_np,
        )
        shared[f"w{l}"] = w_sb
        shared[f"u{l}"] = u_sb
        shared[f"b{l}"] = b_sb
    in_maps = []
    for b in range(B):
        m = dict(shared)
        m["x"] = np.ascontiguousarray(x[b])
        in_maps.append(m)
    return in_maps


def run(inputs, trace=False):
    nc = _get_program()
    in_maps = _make_in_maps(inputs)
    res = run_bass_kernel_spmd(nc, in_maps, list(range(NCORES)), trace=trace)
    out = np.stack([res.results[b]["out"] for b in range(B)], axis=0)
    return out.astype(np.float32), res


def kernel(**inputs):
    out, _ = run(inputs, trace=False)
    return out
